# revision 77
# baseline (speedup 1.0000x reference)
"""Trainium2 Bass kernel for GNN message passing (gather + segment_sum).

reference:
    row, col = edge_index
    out = segment_sum(x[row], col, num_segments=x.shape[0])    # [100000, 128]

Architecture (destination-sharded one-hot-matmul scatter-add + spill path):
- Host: shard destination nodes contiguously across 8 cores (12500/core).
  Per core, dests group into 98 windows of 128 nodes; windows group into
  6 groups of 16 + 1 tail group of 2. Edges bucket by (window, src-chunk)
  (x split into 4 row-quarters so dma_gather's int16 indices reach); each
  bucket keeps its first CAP=208 edges in the main path, the tail spills.
- Main path: per (group, chunk) one dma_gather call pulls 256B fp16 x-rows
  into gather-order slots; window segments are packed back-to-back at CAP
  (not tile-aligned), so some 128-slot tiles straddle two windows. Each
  (tile, window) pair does a one-hot fp16 matmul accumulating into that
  window's PSUM tile (start/stop span the window's 4 chunk sub-blocks).
  Straddle tiles get two masked ohpos columns. PSUM -> fp16 SBUF -> window
  write. Windows of a group live in 16 PSUM tiles (4 banks), double-
  buffered across groups (8 banks total).
- Spill path (~1.5% of edges): per chunk one small gather, then a single
  dma_scatter_add RMWs the fp16 messages straight into the out rows after
  a semaphore confirms every window write has landed (pad slots scatter
  into a sacrificial out row that the host discards).
- Host: concatenate per-core [:12500] out slices, cast fp16 -> f32.
- fp16 end-to-end (x rows, one-hot, matmul in, out rows): segment sums
  accumulate in f32 PSUM; only input rounding + one fp16 RMW per spill
  edge touch the result (~2e-4 rel err vs the 2e-2 gate).
- Capacity overflow (per-(core,chunk) spill > SPILL_CAP, impossible for
  this problem's uniform graph but checked anyway): host fallback computes
  the exact answer instead of returning garbage.
"""

from dataclasses import dataclass

import numpy as np

import jax
from jax.experimental.shard_map import shard_map
from jax.sharding import Mesh, NamedSharding, PartitionSpec

import concourse.bass as bass
import concourse.mybir as mybir
import concourse.tile as tile
from concourse import bass2jax
from concourse.vector_clock import ScopedClock

# ---------------------------------------------------------------------------
# Toolchain workarounds for this walrus build:
# 1) The ISA here allows at most ONE sync-wait command per instruction
#    ("Too many sync wait commands" at codegen otherwise). TileContext's tail
#    drain carries one wait per live semaphore lane, and the scheduler can
#    attach several waits to body instructions too, so every surplus wait is
#    moved onto its own same-engine NOP placed directly before the original
#    instruction (the sequencer executes them in order — semantics identical).
# 2) Extended/pseudo Pool instructions (load_library, dma_gather) need
#    codegen_inst_isa_subclasses before walrus, else "ISA wrong length".
# ---------------------------------------------------------------------------


def _drain_and_barrier_split(self, tick_clock, wait_clock):
    nc = self.nc
    drain_inst = nc.sync.drain()
    wait_clock.add_sem_waits(
        drain_inst.ins, ScopedClock({None: tick_clock.global_clock})
    )
    si = drain_inst.ins.sync_info
    if si is not None and len(si.on_wait) > 0:
        waits = list(si.on_wait)
        si.on_wait = []
        for w in waits:
            nop = nc.sync.nop(nofuse=True)
            nop.ins.sync_info = mybir.SyncInfo(on_wait=[w], on_update=[])
    nc.all_engine_barrier()
    assert self.sems is not None
    popped = nc._tile_sem_poison_stack.pop()
    assert popped is self._sem_poison
    nc.clear_and_free_semaphores(list(self.sems.allocated().values()))
    nc.all_engine_barrier()


tile.TileContext._drain_and_barrier = _drain_and_barrier_split


def split_multi_waits(nc: "bass.Bass", max_waits: int = 1) -> None:
    k = 0
    for fn in nc.m.functions:
        for bb in fn.blocks:
            il = list(bb.instructions)
            out = []
            changed = False
            for inst in il:
                si = inst.sync_info
                if si is not None and len(si.on_wait) > max_waits:
                    waits = list(si.on_wait)
                    si.on_wait = waits[:max_waits]
                    for w in waits[max_waits:]:
                        nop = mybir.InstNoOp(
                            name=f"I-wsplit-{k}", ins=[], outs=[]
                        )
                        k += 1
                        nop.engine = inst.engine
                        nop.sync_info = mybir.SyncInfo(
                            on_wait=[w], on_update=[]
                        )
                        nc.register_instruction(nop, overwrite=True)
                        out.append(nop)
                        changed = True
                out.append(inst)
            if changed:
                bb.instructions = out


# ---------------------------------------------------------------------------
# Kernel
# ---------------------------------------------------------------------------

D = 128
P = 128
N_CORES = 8
NCHUNK = 4  # x row-quarters (int16 dma_gather index reach)


@dataclass(frozen=True)
class Cfg:
    n_nodes: int
    node_per_core: int
    cap: int  # main-path edge capacity per (window, chunk) segment
    group_w: int  # windows per group (PSUM residency set)
    spill_cap: int  # spill slots per (core, chunk), multiple of 128
    gbufs: int = 2

    @property
    def chunk_rows(self) -> int:
        return self.n_nodes // NCHUNK

    @property
    def W(self) -> int:
        return -(-self.node_per_core // P)

    @property
    def n_groups(self) -> int:
        return -(-self.W // self.group_w)

    def group_windows(self, g: int) -> int:
        return min(self.group_w, self.W - g * self.group_w)

    def block_slots(self, g: int) -> int:
        # slots of one (group, chunk) gather call, padded to a tile multiple
        return -(-self.group_windows(g) * self.cap // P) * P

    def block_tiles(self, g: int) -> int:
        return self.block_slots(g) // P

    @property
    def spill_idxs(self) -> int:
        return NCHUNK * self.spill_cap

    @property
    def out_rows(self) -> int:
        return self.W * P


CFG = Cfg(n_nodes=100000, node_per_core=12500, cap=192, group_w=8,
          spill_cap=1280, gbufs=2)


def schedule(cfg: Cfg):
    """Static per-tile schedule: each gathered 128-slot tile gets ONE
    one-hot build (256 lanes wide when the tile straddles two windows) and
    one matmul per covered window. Identical on every core (SPMD).

    Returns (tiles, idx_off, spill_off, scat_off, idx_cols, oh_cols) with
    tiles = [(g, r, t, col, wlo, [(wi, start, stop), ...])]."""
    tiles = []
    col = 0
    for g in range(cfg.n_groups):
        gw = cfg.group_windows(g)
        nt = cfg.block_tiles(g)

        def wins(t):
            lo = (t * P) // cfg.cap
            hi = min((t * P + P - 1) // cfg.cap, gw - 1)
            return list(range(lo, hi + 1))

        touches = [(r, t, wi)
                   for r in range(NCHUNK)
                   for t in range(nt)
                   for wi in wins(t)]
        first = {}
        last = {}
        for k, (r, t, wi) in enumerate(touches):
            first.setdefault(wi, k)
            last[wi] = k
        k = 0
        for r in range(NCHUNK):
            for t in range(nt):
                mm = []
                for wi in wins(t):
                    mm.append((wi, first[wi] == k, last[wi] == k))
                    k += 1
                tiles.append((g, r, t, col, wins(t)[0], mm))
                col += 1
    # per-(g, r) idx-table column offset (int16 cols, wrapped/16)
    idx_off = {}
    off = 0
    for g in range(cfg.n_groups):
        for r in range(NCHUNK):
            idx_off[(g, r)] = off
            off += cfg.block_slots(g) // 16
    # spill gather idx blocks, one per chunk
    spill_off = {}
    for r in range(NCHUNK):
        spill_off[r] = off
        off += cfg.spill_cap // 16
    return tiles, idx_off, spill_off, off, col


def build(cfg: Cfg, reps: int = 1) -> bass.Bass:
    from concourse.library_config import mlp
    from concourse.library_overlay import lower_extended_insts

    tiles_s, idx_off, spill_off, idx_cols, oh_cols = schedule(cfg)
    assert cfg.chunk_rows <= 32767
    nc = bass.Bass(num_swdge_queues=NCHUNK, dynamic_dma_scratch_size=65536)
    x = nc.declare_dram_parameter("x", [cfg.n_nodes, D], mybir.dt.float16,
                                  isOutput=False)
    idxs = nc.declare_dram_parameter("idxs", [P, idx_cols], mybir.dt.int16,
                                     isOutput=False)
    ohpos = nc.declare_dram_parameter("ohpos", [P, oh_cols], mybir.dt.float32,
                                      isOutput=False)
    iota = nc.declare_dram_parameter("iota256", [P, 2 * P], mybir.dt.float16,
                                     isOutput=False)
    # transposed main output: [feature, node] so group writes have 2KB
    # contiguous runs per partition (full DMA rate); host transposes back
    out = nc.declare_dram_parameter("out", [D, cfg.out_rows],
                                    mybir.dt.float16, isOutput=True)
    # spill partial sums land in a separate compact zero-initialized
    # tensor (row = compact spill-dest id); host adds them back. This
    # removes any ordering between window writes and the scatter RMWs.
    spill_out = nc.declare_dram_parameter(
        "spill_out", [NCHUNK * cfg.spill_cap, D], mybir.dt.float16,
        isOutput=True)

    nc.gpsimd.load_library(mlp)
    with tile.TileContext(nc) as tc:
        with (
            tc.tile_pool(name="tabs", bufs=1) as tabs,
            tc.tile_pool(name="gbuf", bufs=cfg.gbufs) as gbuf,
            tc.tile_pool(name="ohb", bufs=4) as ohb,
            tc.tile_pool(name="psumb", bufs=1, space="PSUM") as psumb,
            tc.tile_pool(name="outb", bufs=4) as outb,
        ):
            # idx table loaded as one tile per group (plus one for spill)
            # so the first gather only waits for its own slice
            idxg = {}
            for g in range(cfg.n_groups):
                c0 = idx_off[(g, 0)]
                c1 = idx_off[(g, NCHUNK - 1)] + cfg.block_slots(g) // 16
                t = tabs.tile([P, c1 - c0], mybir.dt.int16,
                              name=f"idxg{g}")
                nc.sync.dma_start(out=t[:], in_=idxs[:, c0:c1])
                idxg[g] = (t, c0)
            c0s = spill_off[0]
            idxsp = tabs.tile([P, idx_cols - c0s], mybir.dt.int16)
            nc.sync.dma_start(out=idxsp[:], in_=idxs[:, c0s:])
            ohpos_sb = tabs.tile([P, oh_cols], mybir.dt.float32)
            iota_sb = tabs.tile([P, 2 * P], mybir.dt.float16)
            spill_sb = tabs.tile([P, (cfg.spill_idxs // P) * D],
                                 mybir.dt.float16)
            nc.sync.dma_start(out=ohpos_sb[:], in_=ohpos[:])
            nc.sync.dma_start(out=iota_sb[:], in_=iota[:])

            # shared registers for gather counts — a fresh to_reg per call
            # exhausts the Pool register file at this call count

            nregs = {}
            for g in range(cfg.n_groups):
                n = cfg.block_slots(g)
                if n not in nregs:
                    nregs[n] = nc.gpsimd.to_reg(n)
            if cfg.spill_cap not in nregs:
                nregs[cfg.spill_cap] = nc.gpsimd.to_reg(cfg.spill_cap)

            by_block = {}
            for (g, r, t, col, wlo, mm) in tiles_s:
                by_block.setdefault((g, r), []).append((t, col, wlo, mm))

            # Tile assigns DMASW sem lanes to Pool DMA insts round-robin in
            # SCHEDULED order (8 lanes), and the runtime locks each lane to
            # the first SWDGE queue that updates it. The scheduler may
            # reorder (it moves the spill gathers, whose only consumer is
            # the final scatter), so the only queue assignment that stays
            # consistent under any order is a single queue.
            def nextq():
                return 0

            for rep in range(reps):
                # spill gathers early (independent of main path)
                for r in range(NCHUNK):
                    t0 = r * (cfg.spill_cap // P)
                    nc.gpsimd.dma_gather(
                        spill_sb[:, t0 * D:(t0 + cfg.spill_cap // P) * D]
                        .rearrange("p (c d) -> p c d", c=cfg.spill_cap // P),
                        x[r * cfg.chunk_rows:(r + 1) * cfg.chunk_rows, :],
                        idxs_sb[:, spill_off[r]:
                                spill_off[r] + cfg.spill_cap // 16],
                        cfg.spill_cap,
                        nregs[cfg.spill_cap],
                        D,
                        queue_num=nextq(),
                        single_packet=False,
                    )
                for g in range(cfg.n_groups):
                    nt = cfg.block_tiles(g)
                    gw = cfg.group_windows(g)
                    chts = []
                    for r in range(NCHUNK):
                        ch = gbuf.tile([P, nt * D], mybir.dt.float16,
                                       tag=f"ch{r}"
                                       if gw == cfg.group_w else f"ch{r}t")
                        c0 = idx_off[(g, r)]
                        nc.gpsimd.dma_gather(
                            ch[:].rearrange("p (c d) -> p c d", c=nt),
                            x[r * cfg.chunk_rows:(r + 1) * cfg.chunk_rows, :],
                            idxs_sb[:, c0:c0 + cfg.block_slots(g) // 16],
                            cfg.block_slots(g),
                            nregs[cfg.block_slots(g)],
                            D,
                            queue_num=nextq(),
                            # single_packet coalesces the whole descriptor
                            # stream into one SDMA packet; beyond ~1K
                            # descriptors that wedges the DMA engine (HW
                            # hang). Multi-packet is required at this size.
                            single_packet=False,
                        )
                        chts.append(ch)
                    # one PSUM tile (= one bank: a matmul `start` zeroes the
                    # whole 2KB zero region) per window, accumulation groups
                    # interleave across banks
                    pstiles = {}
                    obg = outb.tile([P, gw * D], mybir.dt.float16,
                                    tag="obg" if gw == cfg.group_w
                                    else "obgt", name="obg")
                    ndone = 0
                    for r in range(NCHUNK):
                        for (t, col, wlo, mm) in by_block[(g, r)]:
                            wide = 2 if len(mm) > 1 else 1
                            oh = ohb.tile([P, wide * P], mybir.dt.float16,
                                          tag=f"oh{wide}", name="oh")
                            nc.vector.tensor_scalar(
                                out=oh[:],
                                in0=iota_sb[:, :wide * P],
                                scalar1=ohpos_sb[:, col:col + 1],
                                scalar2=None,
                                op0=mybir.AluOpType.is_equal,
                            )
                            for j, (wi, st, sp) in enumerate(mm):
                                if wi not in pstiles:
                                    pstiles[wi] = psumb.tile(
                                        [P, D], mybir.dt.float32,
                                        tag=f"ps{wi}", name=f"ps{wi}")
                                # lhsT = messages, rhs = one-hot: the PSUM
                                # comes out [feature, lane] so the group
                                # write below is a contiguous-run DMA into
                                # the transposed out
                                nc.tensor.matmul(
                                    pstiles[wi][:],
                                    lhsT=chts[r][:, t * D:(t + 1) * D],
                                    rhs=oh[:, j * P:(j + 1) * P],
                                    start=st,
                                    stop=sp,
                                )
                                if sp:
                                    nc.scalar.copy(
                                        out=obg[:, wi * D:(wi + 1) * D],
                                        in_=pstiles[wi][:])
                                    ndone += 1
                    assert ndone == gw
                    row0 = g * cfg.group_w * P
                    nc.sync.dma_start(
                        out=out[:, row0:row0 + gw * P], in_=obg[:]
                    )
                # device-computed spill partials out to the compact DRAM
                # tensor; the host adds them into the final result
                nc.sync.dma_start(
                    out=spill_out[:, :]
                    .rearrange("(c p) d -> p c d", p=P),
                    in_=spill_sb[:]
                    .rearrange("p (c d) -> p c d", c=cfg.spill_idxs // P),
                )
    rebalance_swdge_queues(nc)
    split_multi_waits(nc)
    lower_extended_insts(nc)
    return nc


def rebalance_swdge_queues(nc: "bass.Bass") -> None:
    """Spread SWDGE desc-gen across the 4 Q7 queue pairs.

    All Pool DMAs are emitted on queue 0 because Tile assigns DMASW sem
    lanes round-robin in SCHEDULED order and the runtime locks each lane
    to one queue — emission-order queue cycling breaks when the scheduler
    reorders. After scheduling, each instruction carries its lane
    (bass_scheduled_proc = DMASW<i>), so queue = i % NCHUNK is consistent
    with every lane lock while restoring 4-queue parallel desc-gen on HW.
    Post-drain instructions (the chained spill scatters) have no lane and
    keep queue 0."""
    from concourse.tile_sem_assignment import PROC_NAME_TO_IDX

    lane_q = {
        PROC_NAME_TO_IDX[f"DMASW{i}"]: i % NCHUNK for i in range(8)
    }
    for fn in nc.m.functions:
        for bb in fn.blocks:
            for inst in bb.instructions:
                if isinstance(
                    inst, (mybir.InstDMAGatherAnt, mybir.InstDMAScatterAddAnt)
                ):
                    proc = getattr(inst, "bass_scheduled_proc", None)
                    if proc in lane_q:
                        inst.queue_num = lane_q[proc]


def prep_core(row, col, node_base, cfg: Cfg):
    """Slot assignment for one core.

    Returns (idxs int16, ohpos f32, spill_slots, spill_nodes): the first
    two are device tables; the last two tell the host which spill_out row
    (device-computed message) to add into which local out row."""
    tiles_s, idx_off, spill_off, idx_cols, oh_cols = schedule(cfg)
    lo, hi = node_base, node_base + cfg.node_per_core
    m = (col >= lo) & (col < hi)
    lcol = (col[m] - lo).astype(np.int64)
    lrow = row[m].astype(np.int64)

    w = lcol >> 7
    lane = lcol & 127
    ck = lrow // cfg.chunk_rows
    lidx = lrow - ck * cfg.chunk_rows

    key = w * NCHUNK + ck
    order = np.argsort(key, kind="stable")
    key_s = key[order]
    lane_s = lane[order]
    lidx_s = lidx[order]
    lcol_s = lcol[order]

    nseg = cfg.W * NCHUNK
    counts = np.bincount(key_s, minlength=nseg)
    starts = np.zeros(nseg, np.int64)
    np.cumsum(counts[:-1], out=starts[1:])
    rank = np.arange(len(key_s)) - starts[key_s]

    main = rank < cfg.cap
    # ---- main path tables ----
    w_m = key_s[main] // NCHUNK
    r_m = key_s[main] % NCHUNK
    g_m = w_m // cfg.group_w
    wi_m = w_m - g_m * cfg.group_w
    rank_m = rank[main]
    # slot within the (g, r) call
    slot_m = wi_m * cfg.cap + rank_m

    # per-call int16 source index arrays (slot order i = t*128 + p)
    call_srcidx = {}
    for g in range(cfg.n_groups):
        for r in range(NCHUNK):
            call_srcidx[(g, r)] = np.zeros(cfg.block_slots(g), np.int16)
    # assign
    for g in range(cfg.n_groups):
        for r in range(NCHUNK):
            sel = (g_m == g) & (r_m == r)
            call_srcidx[(g, r)][slot_m[sel]] = lidx_s[main][sel]

    # ohpos [128, oh_cols]: one column per gathered tile; straddle tiles
    # encode lane' = lane + 128 * (window - first window of the tile)
    ohpos = np.full((P, oh_cols), -1.0, np.float32)
    lane_full = {}
    win_full = {}
    for g in range(cfg.n_groups):
        for r in range(NCHUNK):
            ln = np.full(cfg.block_slots(g), -1.0, np.float32)
            wn = np.full(cfg.block_slots(g), -1, np.int64)
            sel = (g_m == g) & (r_m == r)
            ln[slot_m[sel]] = lane_s[main][sel]
            wn[slot_m[sel]] = wi_m[sel]
            lane_full[(g, r)] = ln
            win_full[(g, r)] = wn
    for (g, r, t, colx, wlo, mm) in tiles_s:
        seg = slice(t * P, (t + 1) * P)
        ln = lane_full[(g, r)][seg].copy()
        wn = win_full[(g, r)][seg]
        occ = wn >= 0
        ln[occ] += 128.0 * (wn[occ] - wlo)
        ohpos[:, colx] = ln

    # ---- spill path tables ----
    sp_sel = ~main
    r_sp = key_s[sp_sel] % NCHUNK
    lidx_sp = lidx_s[sp_sel]
    lcol_sp = lcol_s[sp_sel]
    spill_src = {}
    spill_slots = []
    spill_nodes = []
    for r in range(NCHUNK):
        selr = r_sp == r
        n = int(selr.sum())
        if n > cfg.spill_cap:
            raise ValueError(f"spill overflow: {n} > {cfg.spill_cap}")
        s = np.zeros(cfg.spill_cap, np.int16)
        s[:n] = lidx_sp[selr]
        spill_src[r] = s
        spill_slots.append(r * cfg.spill_cap + np.arange(n))
        spill_nodes.append(lcol_sp[selr])
    spill_slots = np.concatenate(spill_slots)
    spill_nodes = np.concatenate(spill_nodes)

    # ---- wrapped int16 idx table ----
    def wrap(flat):
        ci16 = len(flat) // 16
        return np.tile(flat.reshape(ci16, 16).T, (8, 1))

    idxs = np.zeros((P, idx_cols), np.int16)
    for g in range(cfg.n_groups):
        for r in range(NCHUNK):
            c0 = idx_off[(g, r)]
            blk = call_srcidx[(g, r)]
            idxs[:, c0:c0 + len(blk) // 16] = wrap(blk)
    for r in range(NCHUNK):
        c0 = spill_off[r]
        idxs[:, c0:c0 + cfg.spill_cap // 16] = wrap(spill_src[r])
    return idxs, ohpos, spill_slots, spill_nodes


def prep_all(x, edge_index, cfg: Cfg):
    row = np.asarray(edge_index[0])
    col = np.asarray(edge_index[1])
    xf = np.ascontiguousarray(np.asarray(x).astype(np.float16))
    it = np.tile(np.arange(2 * P, dtype=np.float16), (P, 1))
    in_maps = []
    spill_adds = []
    for c in range(N_CORES):
        idxs, ohpos, sslots, snodes = prep_core(
            row, col, c * cfg.node_per_core, cfg)
        in_maps.append({"x": xf, "idxs": idxs, "ohpos": ohpos,
                        "iota256": it})
        spill_adds.append((sslots, snodes))
    return in_maps, spill_adds


class SpmdRunner:
    """PJRT SPMD runner for a prebuilt Bass module.

    Mirrors bass2jax.run_bass_via_pjrt but stages inputs with per-device
    device_put + make_array_from_single_device_arrays and reads outputs
    shard-by-shard: no host<->global-array slicing ops get compiled (this
    toolchain's penguin DataLocalityOpt rejects them for large arrays).
    """

    def __init__(self, nc: bass.Bass, n_cores: int = N_CORES):
        bass2jax.install_neuronx_cc_hook()
        self.nc = nc
        self.n_cores = n_cores
        pname = nc.partition_id_tensor.name if nc.partition_id_tensor else None
        self.partition_name = pname
        in_names, out_names, out_avals = [], [], []
        for alloc in nc.m.functions[0].allocations:
            if not isinstance(alloc, mybir.MemoryLocationSet):
                continue
            name = alloc.memorylocations[0].name
            if alloc.kind == "ExternalInput":
                if name != pname:
                    in_names.append(name)
            elif alloc.kind == "ExternalOutput":
                out_names.append(name)
                out_avals.append(
                    jax.core.ShapedArray(
                        tuple(alloc.tensor_shape), mybir.dt.np(alloc.dtype)
                    )
                )
        self.in_names = in_names
        self.out_names = out_names
        self.out_avals = out_avals
        self.devices = jax.devices()[:n_cores]
        self.mesh = Mesh(np.asarray(self.devices), ("core",))
        self.sharding = NamedSharding(self.mesh, PartitionSpec("core"))
        all_in_names = list(in_names) + list(out_names)
        if pname is not None:
            all_in_names.append(pname)

        def _body(*args):
            operands = list(args)
            if pname is not None:
                operands.append(bass2jax.partition_id_tensor())
            return tuple(
                bass2jax._bass_exec_p.bind(
                    *operands,
                    out_avals=tuple(out_avals),
                    in_names=tuple(all_in_names),
                    out_names=tuple(out_names),
                    lowering_input_output_aliases=(),
                    sim_require_finite=True,
                    sim_require_nnan=True,
                    nc=nc,
                )
            )

        n_args = len(in_names) + len(out_names)
        self.fn = jax.jit(
            shard_map(
                _body,
                mesh=self.mesh,
                in_specs=(PartitionSpec("core"),) * n_args,
                out_specs=(PartitionSpec("core"),) * len(out_names),
                check_rep=False,
            ),
            keep_unused=True,
        )

    def _global(self, per_core_arrays):
        shape = per_core_arrays[0].shape
        gshape = (self.n_cores * shape[0],) + tuple(shape[1:])
        bufs = [
            jax.device_put(a, d)
            for a, d in zip(per_core_arrays, self.devices)
        ]
        return jax.make_array_from_single_device_arrays(
            gshape, self.sharding, bufs
        )

    def stage(self, in_maps):
        args = [
            self._global([np.asarray(m[name]) for m in in_maps])
            for name in self.in_names
        ]
        args += [
            self._global(
                [np.zeros(av.shape, av.dtype) for _ in range(self.n_cores)]
            )
            for av in self.out_avals
        ]
        return args

    def run(self, args):
        outs = self.fn(*args)
        jax.block_until_ready(outs)
        return outs

    def to_numpy(self, outs):
        res = [dict() for _ in range(self.n_cores)]
        for i, name in enumerate(self.out_names):
            shards = sorted(
                outs[i].addressable_shards,
                key=lambda s: s.index[0].start or 0,
            )
            assert len(shards) == self.n_cores
            for c, s in enumerate(shards):
                res[c][name] = np.asarray(s.data)
        return res

    def __call__(self, in_maps):
        return self.to_numpy(self.run(self.stage(in_maps)))


_NC_CACHE = {}
_RUNNER_CACHE = {}


def _get_nc(cfg: Cfg) -> bass.Bass:
    nc = _NC_CACHE.get(cfg)
    if nc is None:
        nc = build(cfg)
        _NC_CACHE[cfg] = nc
    return nc


def _get_runner(cfg: Cfg) -> SpmdRunner:
    r = _RUNNER_CACHE.get(cfg)
    if r is None:
        r = SpmdRunner(_get_nc(cfg))
        _RUNNER_CACHE[cfg] = r
    return r


def kernel(x: np.ndarray, edge_index: np.ndarray) -> np.ndarray:
    x = np.asarray(x)
    edge_index = np.asarray(edge_index)
    try:
        in_maps, spill_adds = prep_all(x, edge_index, CFG)
    except ValueError:
        # Spill-capacity overflow (an edge distribution far from this
        # problem's uniform random graph): fall back to a host computation
        # rather than returning wrong results.
        out = np.zeros((x.shape[0], x.shape[1]), np.float32)
        np.add.at(
            out,
            np.asarray(edge_index[1], np.int64),
            np.asarray(x, np.float32)[np.asarray(edge_index[0], np.int64)],
        )
        return out
    res = _get_runner(CFG)(in_maps)
    parts = []
    for c in range(N_CORES):
        out_c = np.ascontiguousarray(
            res[c]["out"].T[: CFG.node_per_core]).astype(np.float32)
        sslots, snodes = spill_adds[c]
        if len(snodes):
            vals = res[c]["spill_out"].astype(np.float32)
            # device-computed spill messages merged by duplicate-rank round
            # so each fancy-index add has unique destination rows
            o = np.argsort(snodes, kind="stable")
            ns, ss = snodes[o], sslots[o]
            grp_start = np.ones(len(ns), bool)
            grp_start[1:] = ns[1:] != ns[:-1]
            gid = np.cumsum(grp_start) - 1
            firsts = np.nonzero(grp_start)[0]
            rnd = np.arange(len(ns)) - firsts[gid]
            for k in range(int(rnd.max()) + 1 if len(ns) else 0):
                selk = rnd == k
                out_c[ns[selk]] += vals[ss[selk]]
        parts.append(out_c)
    return np.concatenate(parts)


# revision 79
# speedup vs baseline: 2.0778x; 2.0778x over previous
"""Trainium2 Bass kernel for GNN message passing (gather + segment_sum).

reference:
    row, col = edge_index
    out = segment_sum(x[row], col, num_segments=x.shape[0])    # [100000, 128]

Architecture (destination-sharded one-hot-matmul scatter-add + spill path):
- Host: shard destination nodes contiguously across 8 cores (12500/core).
  Per core, dests group into 98 windows of 128 nodes; windows group into
  6 groups of 16 + 1 tail group of 2. Edges bucket by (window, src-chunk)
  (x split into 4 row-quarters so dma_gather's int16 indices reach); each
  bucket keeps its first CAP=208 edges in the main path, the tail spills.
- Main path: per (group, chunk) one dma_gather call pulls 256B fp16 x-rows
  into gather-order slots; window segments are packed back-to-back at CAP
  (not tile-aligned), so some 128-slot tiles straddle two windows. Each
  (tile, window) pair does a one-hot fp16 matmul accumulating into that
  window's PSUM tile (start/stop span the window's 4 chunk sub-blocks).
  Straddle tiles get two masked ohpos columns. PSUM -> fp16 SBUF -> window
  write. Windows of a group live in 16 PSUM tiles (4 banks), double-
  buffered across groups (8 banks total).
- Spill path (~1.5% of edges): per chunk one small gather, then a single
  dma_scatter_add RMWs the fp16 messages straight into the out rows after
  a semaphore confirms every window write has landed (pad slots scatter
  into a sacrificial out row that the host discards).
- Host: concatenate per-core [:12500] out slices, cast fp16 -> f32.
- fp16 end-to-end (x rows, one-hot, matmul in, out rows): segment sums
  accumulate in f32 PSUM; only input rounding + one fp16 RMW per spill
  edge touch the result (~2e-4 rel err vs the 2e-2 gate).
- Capacity overflow (per-(core,chunk) spill > SPILL_CAP, impossible for
  this problem's uniform graph but checked anyway): host fallback computes
  the exact answer instead of returning garbage.
"""

from dataclasses import dataclass

import numpy as np

import jax
from jax.experimental.shard_map import shard_map
from jax.sharding import Mesh, NamedSharding, PartitionSpec

import concourse.bass as bass
import concourse.mybir as mybir
import concourse.tile as tile
from concourse import bass2jax
from concourse.vector_clock import ScopedClock

# ---------------------------------------------------------------------------
# Toolchain workarounds for this walrus build:
# 1) The ISA here allows at most ONE sync-wait command per instruction
#    ("Too many sync wait commands" at codegen otherwise). TileContext's tail
#    drain carries one wait per live semaphore lane, and the scheduler can
#    attach several waits to body instructions too, so every surplus wait is
#    moved onto its own same-engine NOP placed directly before the original
#    instruction (the sequencer executes them in order — semantics identical).
# 2) Extended/pseudo Pool instructions (load_library, dma_gather) need
#    codegen_inst_isa_subclasses before walrus, else "ISA wrong length".
# ---------------------------------------------------------------------------


def _drain_and_barrier_split(self, tick_clock, wait_clock):
    nc = self.nc
    drain_inst = nc.sync.drain()
    wait_clock.add_sem_waits(
        drain_inst.ins, ScopedClock({None: tick_clock.global_clock})
    )
    si = drain_inst.ins.sync_info
    if si is not None and len(si.on_wait) > 0:
        waits = list(si.on_wait)
        si.on_wait = []
        for w in waits:
            nop = nc.sync.nop(nofuse=True)
            nop.ins.sync_info = mybir.SyncInfo(on_wait=[w], on_update=[])
    nc.all_engine_barrier()
    assert self.sems is not None
    popped = nc._tile_sem_poison_stack.pop()
    assert popped is self._sem_poison
    nc.clear_and_free_semaphores(list(self.sems.allocated().values()))
    nc.all_engine_barrier()


tile.TileContext._drain_and_barrier = _drain_and_barrier_split


def split_multi_waits(nc: "bass.Bass", max_waits: int = 1) -> None:
    k = 0
    for fn in nc.m.functions:
        for bb in fn.blocks:
            il = list(bb.instructions)
            out = []
            changed = False
            for inst in il:
                si = inst.sync_info
                if si is not None and len(si.on_wait) > max_waits:
                    waits = list(si.on_wait)
                    si.on_wait = waits[:max_waits]
                    for w in waits[max_waits:]:
                        nop = mybir.InstNoOp(
                            name=f"I-wsplit-{k}", ins=[], outs=[]
                        )
                        k += 1
                        nop.engine = inst.engine
                        nop.sync_info = mybir.SyncInfo(
                            on_wait=[w], on_update=[]
                        )
                        nc.register_instruction(nop, overwrite=True)
                        out.append(nop)
                        changed = True
                out.append(inst)
            if changed:
                bb.instructions = out


# ---------------------------------------------------------------------------
# Kernel
# ---------------------------------------------------------------------------

D = 128
P = 128
N_CORES = 8
NCHUNK = 4  # x row-quarters (int16 dma_gather index reach)


@dataclass(frozen=True)
class Cfg:
    n_nodes: int
    node_per_core: int
    cap: int  # main-path edge capacity per (window, chunk) segment
    group_w: int  # windows per group (PSUM residency set)
    spill_cap: int  # spill slots per (core, chunk), multiple of 128
    gbufs: int = 2

    @property
    def chunk_rows(self) -> int:
        return self.n_nodes // NCHUNK

    @property
    def W(self) -> int:
        return -(-self.node_per_core // P)

    @property
    def n_groups(self) -> int:
        return -(-self.W // self.group_w)

    def group_windows(self, g: int) -> int:
        return min(self.group_w, self.W - g * self.group_w)

    def block_slots(self, g: int) -> int:
        # slots of one (group, chunk) gather call, padded to a tile multiple
        return -(-self.group_windows(g) * self.cap // P) * P

    def block_tiles(self, g: int) -> int:
        return self.block_slots(g) // P

    @property
    def spill_idxs(self) -> int:
        return NCHUNK * self.spill_cap

    @property
    def out_rows(self) -> int:
        return self.W * P


CFG = Cfg(n_nodes=100000, node_per_core=12500, cap=192, group_w=8,
          spill_cap=1280, gbufs=2)


def schedule(cfg: Cfg):
    """Static per-tile schedule: each gathered 128-slot tile gets ONE
    one-hot build (256 lanes wide when the tile straddles two windows) and
    one matmul per covered window. Identical on every core (SPMD).

    Returns (tiles, idx_off, spill_off, scat_off, idx_cols, oh_cols) with
    tiles = [(g, r, t, col, wlo, [(wi, start, stop), ...])]."""
    tiles = []
    col = 0
    for g in range(cfg.n_groups):
        gw = cfg.group_windows(g)
        nt = cfg.block_tiles(g)

        def wins(t):
            lo = (t * P) // cfg.cap
            hi = min((t * P + P - 1) // cfg.cap, gw - 1)
            return list(range(lo, hi + 1))

        touches = [(r, t, wi)
                   for r in range(NCHUNK)
                   for t in range(nt)
                   for wi in wins(t)]
        first = {}
        last = {}
        for k, (r, t, wi) in enumerate(touches):
            first.setdefault(wi, k)
            last[wi] = k
        k = 0
        for r in range(NCHUNK):
            for t in range(nt):
                mm = []
                for wi in wins(t):
                    mm.append((wi, first[wi] == k, last[wi] == k))
                    k += 1
                tiles.append((g, r, t, col, wins(t)[0], mm))
                col += 1
    # per-(g, r) idx-table column offset (int16 cols, wrapped/16)
    idx_off = {}
    off = 0
    for g in range(cfg.n_groups):
        for r in range(NCHUNK):
            idx_off[(g, r)] = off
            off += cfg.block_slots(g) // 16
    # spill gather idx blocks, one per chunk
    spill_off = {}
    for r in range(NCHUNK):
        spill_off[r] = off
        off += cfg.spill_cap // 16
    return tiles, idx_off, spill_off, off, col


def build(cfg: Cfg, reps: int = 1) -> bass.Bass:
    from concourse.library_config import mlp
    from concourse.library_overlay import lower_extended_insts

    tiles_s, idx_off, spill_off, idx_cols, oh_cols = schedule(cfg)
    assert cfg.chunk_rows <= 32767
    nc = bass.Bass(num_swdge_queues=NCHUNK, dynamic_dma_scratch_size=65536)
    x = nc.declare_dram_parameter("x", [cfg.n_nodes, D], mybir.dt.float16,
                                  isOutput=False)
    idxs = nc.declare_dram_parameter("idxs", [P, idx_cols], mybir.dt.int16,
                                     isOutput=False)
    ohpos = nc.declare_dram_parameter("ohpos", [P, oh_cols], mybir.dt.float32,
                                      isOutput=False)
    iota = nc.declare_dram_parameter("iota256", [P, 2 * P], mybir.dt.float16,
                                     isOutput=False)
    # transposed main output: [feature, node] so group writes have 2KB
    # contiguous runs per partition (full DMA rate); host transposes back
    out = nc.declare_dram_parameter("out", [D, cfg.out_rows],
                                    mybir.dt.float16, isOutput=True)
    # spill partial sums land in a separate compact zero-initialized
    # tensor (row = compact spill-dest id); host adds them back. This
    # removes any ordering between window writes and the scatter RMWs.
    spill_out = nc.declare_dram_parameter(
        "spill_out", [NCHUNK * cfg.spill_cap, D], mybir.dt.float16,
        isOutput=True)

    nc.gpsimd.load_library(mlp)
    with tile.TileContext(nc) as tc:
        with (
            tc.tile_pool(name="tabs", bufs=1) as tabs,
            tc.tile_pool(name="gbuf", bufs=cfg.gbufs) as gbuf,
            tc.tile_pool(name="ohb", bufs=4) as ohb,
            tc.tile_pool(name="psumb", bufs=1, space="PSUM") as psumb,
            tc.tile_pool(name="outb", bufs=4) as outb,
        ):
            # idx table loaded as one tile per group (plus one for spill)
            # so the first gather only waits for its own slice
            idxg = {}
            for g in range(cfg.n_groups):
                c0 = idx_off[(g, 0)]
                c1 = idx_off[(g, NCHUNK - 1)] + cfg.block_slots(g) // 16
                t = tabs.tile([P, c1 - c0], mybir.dt.int16,
                              name=f"idxg{g}")
                nc.sync.dma_start(out=t[:], in_=idxs[:, c0:c1])
                idxg[g] = (t, c0)
            c0s = spill_off[0]
            idxsp = tabs.tile([P, idx_cols - c0s], mybir.dt.int16)
            nc.sync.dma_start(out=idxsp[:], in_=idxs[:, c0s:])
            ohpos_sb = tabs.tile([P, oh_cols], mybir.dt.float32)
            iota_sb = tabs.tile([P, 2 * P], mybir.dt.float16)
            spill_sb = tabs.tile([P, (cfg.spill_idxs // P) * D],
                                 mybir.dt.float16)
            nc.sync.dma_start(out=ohpos_sb[:], in_=ohpos[:])
            nc.sync.dma_start(out=iota_sb[:], in_=iota[:])

            # shared registers for gather counts — a fresh to_reg per call
            # exhausts the Pool register file at this call count

            nregs = {}
            for g in range(cfg.n_groups):
                n = cfg.block_slots(g)
                if n not in nregs:
                    nregs[n] = nc.gpsimd.to_reg(n)
            if cfg.spill_cap not in nregs:
                nregs[cfg.spill_cap] = nc.gpsimd.to_reg(cfg.spill_cap)

            by_block = {}
            for (g, r, t, col, wlo, mm) in tiles_s:
                by_block.setdefault((g, r), []).append((t, col, wlo, mm))

            # Tile assigns DMASW sem lanes to Pool DMA insts round-robin in
            # SCHEDULED order (8 lanes), and the runtime locks each lane to
            # the first SWDGE queue that updates it. The scheduler may
            # reorder (it moves the spill gathers, whose only consumer is
            # the final scatter), so the only queue assignment that stays
            # consistent under any order is a single queue.
            def nextq():
                return 0

            for rep in range(reps):
                # spill gathers early (independent of main path)
                for r in range(NCHUNK):
                    t0 = r * (cfg.spill_cap // P)
                    nc.gpsimd.dma_gather(
                        spill_sb[:, t0 * D:(t0 + cfg.spill_cap // P) * D]
                        .rearrange("p (c d) -> p c d", c=cfg.spill_cap // P),
                        x[r * cfg.chunk_rows:(r + 1) * cfg.chunk_rows, :],
                        idxsp[:, spill_off[r] - c0s:
                              spill_off[r] - c0s + cfg.spill_cap // 16],
                        cfg.spill_cap,
                        nregs[cfg.spill_cap],
                        D,
                        queue_num=nextq(),
                        single_packet=False,
                    )
                for g in range(cfg.n_groups):
                    nt = cfg.block_tiles(g)
                    gw = cfg.group_windows(g)
                    chts = []
                    for r in range(NCHUNK):
                        ch = gbuf.tile([P, nt * D], mybir.dt.float16,
                                       tag=f"ch{r}"
                                       if gw == cfg.group_w else f"ch{r}t")
                        c0 = idx_off[(g, r)]
                        nc.gpsimd.dma_gather(
                            ch[:].rearrange("p (c d) -> p c d", c=nt),
                            x[r * cfg.chunk_rows:(r + 1) * cfg.chunk_rows, :],
                            idxg[g][0][:, c0 - idxg[g][1]:
                                       c0 - idxg[g][1]
                                       + cfg.block_slots(g) // 16],
                            cfg.block_slots(g),
                            nregs[cfg.block_slots(g)],
                            D,
                            queue_num=nextq(),
                            # single_packet coalesces the whole descriptor
                            # stream into one SDMA packet; beyond ~1K
                            # descriptors that wedges the DMA engine (HW
                            # hang). Multi-packet is required at this size.
                            single_packet=False,
                        )
                        chts.append(ch)
                    # one PSUM tile (= one bank: a matmul `start` zeroes the
                    # whole 2KB zero region) per window, accumulation groups
                    # interleave across banks
                    pstiles = {}
                    obg = outb.tile([P, gw * D], mybir.dt.float16,
                                    tag="obg" if gw == cfg.group_w
                                    else "obgt", name="obg")
                    ndone = 0
                    for r in range(NCHUNK):
                        for (t, col, wlo, mm) in by_block[(g, r)]:
                            wide = 2 if len(mm) > 1 else 1
                            oh = ohb.tile([P, wide * P], mybir.dt.float16,
                                          tag=f"oh{wide}", name="oh")
                            nc.vector.tensor_scalar(
                                out=oh[:],
                                in0=iota_sb[:, :wide * P],
                                scalar1=ohpos_sb[:, col:col + 1],
                                scalar2=None,
                                op0=mybir.AluOpType.is_equal,
                            )
                            for j, (wi, st, sp) in enumerate(mm):
                                if wi not in pstiles:
                                    pstiles[wi] = psumb.tile(
                                        [P, D], mybir.dt.float32,
                                        tag=f"ps{wi}", name=f"ps{wi}")
                                # lhsT = messages, rhs = one-hot: the PSUM
                                # comes out [feature, lane] so the group
                                # write below is a contiguous-run DMA into
                                # the transposed out
                                nc.tensor.matmul(
                                    pstiles[wi][:],
                                    lhsT=chts[r][:, t * D:(t + 1) * D],
                                    rhs=oh[:, j * P:(j + 1) * P],
                                    start=st,
                                    stop=sp,
                                )
                                if sp:
                                    nc.scalar.copy(
                                        out=obg[:, wi * D:(wi + 1) * D],
                                        in_=pstiles[wi][:])
                                    ndone += 1
                    assert ndone == gw
                    row0 = g * cfg.group_w * P
                    nc.sync.dma_start(
                        out=out[:, row0:row0 + gw * P], in_=obg[:]
                    )
                # device-computed spill partials out to the compact DRAM
                # tensor; the host adds them into the final result
                nc.sync.dma_start(
                    out=spill_out[:, :]
                    .rearrange("(c p) d -> p c d", p=P),
                    in_=spill_sb[:]
                    .rearrange("p (c d) -> p c d", c=cfg.spill_idxs // P),
                )
    rebalance_swdge_queues(nc)
    split_multi_waits(nc)
    lower_extended_insts(nc)
    return nc


def rebalance_swdge_queues(nc: "bass.Bass") -> None:
    """Spread SWDGE desc-gen across the 4 Q7 queue pairs.

    All Pool DMAs are emitted on queue 0 because Tile assigns DMASW sem
    lanes round-robin in SCHEDULED order and the runtime locks each lane
    to one queue — emission-order queue cycling breaks when the scheduler
    reorders. After scheduling, each instruction carries its lane
    (bass_scheduled_proc = DMASW<i>), so queue = i % NCHUNK is consistent
    with every lane lock while restoring 4-queue parallel desc-gen on HW.
    Post-drain instructions (the chained spill scatters) have no lane and
    keep queue 0."""
    from concourse.tile_sem_assignment import PROC_NAME_TO_IDX

    lane_q = {
        PROC_NAME_TO_IDX[f"DMASW{i}"]: i % NCHUNK for i in range(8)
    }
    for fn in nc.m.functions:
        for bb in fn.blocks:
            for inst in bb.instructions:
                if isinstance(
                    inst, (mybir.InstDMAGatherAnt, mybir.InstDMAScatterAddAnt)
                ):
                    proc = getattr(inst, "bass_scheduled_proc", None)
                    if proc in lane_q:
                        inst.queue_num = lane_q[proc]


def prep_core(row, col, node_base, cfg: Cfg):
    """Slot assignment for one core.

    Returns (idxs int16, ohpos f32, spill_slots, spill_nodes): the first
    two are device tables; the last two tell the host which spill_out row
    (device-computed message) to add into which local out row."""
    tiles_s, idx_off, spill_off, idx_cols, oh_cols = schedule(cfg)
    lo, hi = node_base, node_base + cfg.node_per_core
    m = (col >= lo) & (col < hi)
    lcol = (col[m] - lo).astype(np.int64)
    lrow = row[m].astype(np.int64)

    w = lcol >> 7
    lane = lcol & 127
    ck = lrow // cfg.chunk_rows
    lidx = lrow - ck * cfg.chunk_rows

    key = w * NCHUNK + ck
    order = np.argsort(key, kind="stable")
    key_s = key[order]
    lane_s = lane[order]
    lidx_s = lidx[order]
    lcol_s = lcol[order]

    nseg = cfg.W * NCHUNK
    counts = np.bincount(key_s, minlength=nseg)
    starts = np.zeros(nseg, np.int64)
    np.cumsum(counts[:-1], out=starts[1:])
    rank = np.arange(len(key_s)) - starts[key_s]

    main = rank < cfg.cap
    # ---- main path tables ----
    w_m = key_s[main] // NCHUNK
    r_m = key_s[main] % NCHUNK
    g_m = w_m // cfg.group_w
    wi_m = w_m - g_m * cfg.group_w
    rank_m = rank[main]
    # slot within the (g, r) call
    slot_m = wi_m * cfg.cap + rank_m

    # per-call int16 source index arrays (slot order i = t*128 + p)
    call_srcidx = {}
    for g in range(cfg.n_groups):
        for r in range(NCHUNK):
            call_srcidx[(g, r)] = np.zeros(cfg.block_slots(g), np.int16)
    # assign
    for g in range(cfg.n_groups):
        for r in range(NCHUNK):
            sel = (g_m == g) & (r_m == r)
            call_srcidx[(g, r)][slot_m[sel]] = lidx_s[main][sel]

    # ohpos [128, oh_cols]: one column per gathered tile; straddle tiles
    # encode lane' = lane + 128 * (window - first window of the tile)
    ohpos = np.full((P, oh_cols), -1.0, np.float32)
    lane_full = {}
    win_full = {}
    for g in range(cfg.n_groups):
        for r in range(NCHUNK):
            ln = np.full(cfg.block_slots(g), -1.0, np.float32)
            wn = np.full(cfg.block_slots(g), -1, np.int64)
            sel = (g_m == g) & (r_m == r)
            ln[slot_m[sel]] = lane_s[main][sel]
            wn[slot_m[sel]] = wi_m[sel]
            lane_full[(g, r)] = ln
            win_full[(g, r)] = wn
    for (g, r, t, colx, wlo, mm) in tiles_s:
        seg = slice(t * P, (t + 1) * P)
        ln = lane_full[(g, r)][seg].copy()
        wn = win_full[(g, r)][seg]
        occ = wn >= 0
        ln[occ] += 128.0 * (wn[occ] - wlo)
        ohpos[:, colx] = ln

    # ---- spill path tables ----
    sp_sel = ~main
    r_sp = key_s[sp_sel] % NCHUNK
    lidx_sp = lidx_s[sp_sel]
    lcol_sp = lcol_s[sp_sel]
    spill_src = {}
    spill_slots = []
    spill_nodes = []
    for r in range(NCHUNK):
        selr = r_sp == r
        n = int(selr.sum())
        if n > cfg.spill_cap:
            raise ValueError(f"spill overflow: {n} > {cfg.spill_cap}")
        s = np.zeros(cfg.spill_cap, np.int16)
        s[:n] = lidx_sp[selr]
        spill_src[r] = s
        spill_slots.append(r * cfg.spill_cap + np.arange(n))
        spill_nodes.append(lcol_sp[selr])
    spill_slots = np.concatenate(spill_slots)
    spill_nodes = np.concatenate(spill_nodes)

    # ---- wrapped int16 idx table ----
    def wrap(flat):
        ci16 = len(flat) // 16
        return np.tile(flat.reshape(ci16, 16).T, (8, 1))

    idxs = np.zeros((P, idx_cols), np.int16)
    for g in range(cfg.n_groups):
        for r in range(NCHUNK):
            c0 = idx_off[(g, r)]
            blk = call_srcidx[(g, r)]
            idxs[:, c0:c0 + len(blk) // 16] = wrap(blk)
    for r in range(NCHUNK):
        c0 = spill_off[r]
        idxs[:, c0:c0 + cfg.spill_cap // 16] = wrap(spill_src[r])
    return idxs, ohpos, spill_slots, spill_nodes


def prep_all(x, edge_index, cfg: Cfg):
    row = np.asarray(edge_index[0])
    col = np.asarray(edge_index[1])
    xf = np.ascontiguousarray(np.asarray(x).astype(np.float16))
    it = np.tile(np.arange(2 * P, dtype=np.float16), (P, 1))
    in_maps = []
    spill_adds = []
    for c in range(N_CORES):
        idxs, ohpos, sslots, snodes = prep_core(
            row, col, c * cfg.node_per_core, cfg)
        in_maps.append({"x": xf, "idxs": idxs, "ohpos": ohpos,
                        "iota256": it})
        spill_adds.append((sslots, snodes))
    return in_maps, spill_adds


class SpmdRunner:
    """PJRT SPMD runner for a prebuilt Bass module.

    Mirrors bass2jax.run_bass_via_pjrt but stages inputs with per-device
    device_put + make_array_from_single_device_arrays and reads outputs
    shard-by-shard: no host<->global-array slicing ops get compiled (this
    toolchain's penguin DataLocalityOpt rejects them for large arrays).
    """

    def __init__(self, nc: bass.Bass, n_cores: int = N_CORES):
        bass2jax.install_neuronx_cc_hook()
        self.nc = nc
        self.n_cores = n_cores
        pname = nc.partition_id_tensor.name if nc.partition_id_tensor else None
        self.partition_name = pname
        in_names, out_names, out_avals = [], [], []
        for alloc in nc.m.functions[0].allocations:
            if not isinstance(alloc, mybir.MemoryLocationSet):
                continue
            name = alloc.memorylocations[0].name
            if alloc.kind == "ExternalInput":
                if name != pname:
                    in_names.append(name)
            elif alloc.kind == "ExternalOutput":
                out_names.append(name)
                out_avals.append(
                    jax.core.ShapedArray(
                        tuple(alloc.tensor_shape), mybir.dt.np(alloc.dtype)
                    )
                )
        self.in_names = in_names
        self.out_names = out_names
        self.out_avals = out_avals
        self.devices = jax.devices()[:n_cores]
        self.mesh = Mesh(np.asarray(self.devices), ("core",))
        self.sharding = NamedSharding(self.mesh, PartitionSpec("core"))
        all_in_names = list(in_names) + list(out_names)
        if pname is not None:
            all_in_names.append(pname)

        def _body(*args):
            operands = list(args)
            if pname is not None:
                operands.append(bass2jax.partition_id_tensor())
            return tuple(
                bass2jax._bass_exec_p.bind(
                    *operands,
                    out_avals=tuple(out_avals),
                    in_names=tuple(all_in_names),
                    out_names=tuple(out_names),
                    lowering_input_output_aliases=(),
                    sim_require_finite=True,
                    sim_require_nnan=True,
                    nc=nc,
                )
            )

        n_args = len(in_names) + len(out_names)
        self.fn = jax.jit(
            shard_map(
                _body,
                mesh=self.mesh,
                in_specs=(PartitionSpec("core"),) * n_args,
                out_specs=(PartitionSpec("core"),) * len(out_names),
                check_rep=False,
            ),
            keep_unused=True,
        )

    def _global(self, per_core_arrays):
        shape = per_core_arrays[0].shape
        gshape = (self.n_cores * shape[0],) + tuple(shape[1:])
        bufs = [
            jax.device_put(a, d)
            for a, d in zip(per_core_arrays, self.devices)
        ]
        return jax.make_array_from_single_device_arrays(
            gshape, self.sharding, bufs
        )

    def stage(self, in_maps):
        args = [
            self._global([np.asarray(m[name]) for m in in_maps])
            for name in self.in_names
        ]
        args += [
            self._global(
                [np.zeros(av.shape, av.dtype) for _ in range(self.n_cores)]
            )
            for av in self.out_avals
        ]
        return args

    def run(self, args):
        outs = self.fn(*args)
        jax.block_until_ready(outs)
        return outs

    def to_numpy(self, outs):
        res = [dict() for _ in range(self.n_cores)]
        for i, name in enumerate(self.out_names):
            shards = sorted(
                outs[i].addressable_shards,
                key=lambda s: s.index[0].start or 0,
            )
            assert len(shards) == self.n_cores
            for c, s in enumerate(shards):
                res[c][name] = np.asarray(s.data)
        return res

    def __call__(self, in_maps):
        return self.to_numpy(self.run(self.stage(in_maps)))


_NC_CACHE = {}
_RUNNER_CACHE = {}


def _get_nc(cfg: Cfg) -> bass.Bass:
    nc = _NC_CACHE.get(cfg)
    if nc is None:
        nc = build(cfg)
        _NC_CACHE[cfg] = nc
    return nc


def _get_runner(cfg: Cfg) -> SpmdRunner:
    r = _RUNNER_CACHE.get(cfg)
    if r is None:
        r = SpmdRunner(_get_nc(cfg))
        _RUNNER_CACHE[cfg] = r
    return r


def kernel(x: np.ndarray, edge_index: np.ndarray) -> np.ndarray:
    x = np.asarray(x)
    edge_index = np.asarray(edge_index)
    try:
        in_maps, spill_adds = prep_all(x, edge_index, CFG)
    except ValueError:
        # Spill-capacity overflow (an edge distribution far from this
        # problem's uniform random graph): fall back to a host computation
        # rather than returning wrong results.
        out = np.zeros((x.shape[0], x.shape[1]), np.float32)
        np.add.at(
            out,
            np.asarray(edge_index[1], np.int64),
            np.asarray(x, np.float32)[np.asarray(edge_index[0], np.int64)],
        )
        return out
    res = _get_runner(CFG)(in_maps)
    parts = []
    for c in range(N_CORES):
        out_c = np.ascontiguousarray(
            res[c]["out"].T[: CFG.node_per_core]).astype(np.float32)
        sslots, snodes = spill_adds[c]
        if len(snodes):
            vals = res[c]["spill_out"].astype(np.float32)
            # device-computed spill messages merged by duplicate-rank round
            # so each fancy-index add has unique destination rows
            o = np.argsort(snodes, kind="stable")
            ns, ss = snodes[o], sslots[o]
            grp_start = np.ones(len(ns), bool)
            grp_start[1:] = ns[1:] != ns[:-1]
            gid = np.cumsum(grp_start) - 1
            firsts = np.nonzero(grp_start)[0]
            rnd = np.arange(len(ns)) - firsts[gid]
            for k in range(int(rnd.max()) + 1 if len(ns) else 0):
                selk = rnd == k
                out_c[ns[selk]] += vals[ss[selk]]
        parts.append(out_c)
    return np.concatenate(parts)


# revision 82
# speedup vs baseline: 2.0825x; 1.0023x over previous
"""Trainium2 Bass kernel for GNN message passing (gather + segment_sum).

reference:
    row, col = edge_index
    out = segment_sum(x[row], col, num_segments=x.shape[0])    # [100000, 128]

Architecture (destination-sharded one-hot-matmul scatter-add + spill path):
- Host: shard destination nodes contiguously across 8 cores (12500/core).
  Per core, dests group into 98 windows of 128 nodes; windows group into
  6 groups of 16 + 1 tail group of 2. Edges bucket by (window, src-chunk)
  (x split into 4 row-quarters so dma_gather's int16 indices reach); each
  bucket keeps its first CAP=208 edges in the main path, the tail spills.
- Main path: per (group, chunk) one dma_gather call pulls 256B fp16 x-rows
  into gather-order slots; window segments are packed back-to-back at CAP
  (not tile-aligned), so some 128-slot tiles straddle two windows. Each
  (tile, window) pair does a one-hot fp16 matmul accumulating into that
  window's PSUM tile (start/stop span the window's 4 chunk sub-blocks).
  Straddle tiles get two masked ohpos columns. PSUM -> fp16 SBUF -> window
  write. Windows of a group live in 16 PSUM tiles (4 banks), double-
  buffered across groups (8 banks total).
- Spill path (~1.5% of edges): per chunk one small gather, then a single
  dma_scatter_add RMWs the fp16 messages straight into the out rows after
  a semaphore confirms every window write has landed (pad slots scatter
  into a sacrificial out row that the host discards).
- Host: concatenate per-core [:12500] out slices, cast fp16 -> f32.
- fp16 end-to-end (x rows, one-hot, matmul in, out rows): segment sums
  accumulate in f32 PSUM; only input rounding + one fp16 RMW per spill
  edge touch the result (~2e-4 rel err vs the 2e-2 gate).
- Capacity overflow (per-(core,chunk) spill > SPILL_CAP, impossible for
  this problem's uniform graph but checked anyway): host fallback computes
  the exact answer instead of returning garbage.
"""

from dataclasses import dataclass

import numpy as np

import jax
from jax.experimental.shard_map import shard_map
from jax.sharding import Mesh, NamedSharding, PartitionSpec

import concourse.bass as bass
import concourse.mybir as mybir
import concourse.tile as tile
from concourse import bass2jax
from concourse.vector_clock import ScopedClock

# ---------------------------------------------------------------------------
# Toolchain workarounds for this walrus build:
# 1) The ISA here allows at most ONE sync-wait command per instruction
#    ("Too many sync wait commands" at codegen otherwise). TileContext's tail
#    drain carries one wait per live semaphore lane, and the scheduler can
#    attach several waits to body instructions too, so every surplus wait is
#    moved onto its own same-engine NOP placed directly before the original
#    instruction (the sequencer executes them in order — semantics identical).
# 2) Extended/pseudo Pool instructions (load_library, dma_gather) need
#    codegen_inst_isa_subclasses before walrus, else "ISA wrong length".
# ---------------------------------------------------------------------------


def _drain_and_barrier_split(self, tick_clock, wait_clock):
    nc = self.nc
    drain_inst = nc.sync.drain()
    wait_clock.add_sem_waits(
        drain_inst.ins, ScopedClock({None: tick_clock.global_clock})
    )
    si = drain_inst.ins.sync_info
    if si is not None and len(si.on_wait) > 0:
        waits = list(si.on_wait)
        si.on_wait = []
        for w in waits:
            nop = nc.sync.nop(nofuse=True)
            nop.ins.sync_info = mybir.SyncInfo(on_wait=[w], on_update=[])
    nc.all_engine_barrier()
    assert self.sems is not None
    popped = nc._tile_sem_poison_stack.pop()
    assert popped is self._sem_poison
    nc.clear_and_free_semaphores(list(self.sems.allocated().values()))
    nc.all_engine_barrier()


tile.TileContext._drain_and_barrier = _drain_and_barrier_split


def split_multi_waits(nc: "bass.Bass", max_waits: int = 1) -> None:
    k = 0
    for fn in nc.m.functions:
        for bb in fn.blocks:
            il = list(bb.instructions)
            out = []
            changed = False
            for inst in il:
                si = inst.sync_info
                if si is not None and len(si.on_wait) > max_waits:
                    waits = list(si.on_wait)
                    si.on_wait = waits[:max_waits]
                    for w in waits[max_waits:]:
                        nop = mybir.InstNoOp(
                            name=f"I-wsplit-{k}", ins=[], outs=[]
                        )
                        k += 1
                        nop.engine = inst.engine
                        nop.sync_info = mybir.SyncInfo(
                            on_wait=[w], on_update=[]
                        )
                        nc.register_instruction(nop, overwrite=True)
                        out.append(nop)
                        changed = True
                out.append(inst)
            if changed:
                bb.instructions = out


# ---------------------------------------------------------------------------
# Kernel
# ---------------------------------------------------------------------------

D = 128
P = 128
N_CORES = 8
NCHUNK = 4  # x row-quarters (int16 dma_gather index reach)


@dataclass(frozen=True)
class Cfg:
    n_nodes: int
    node_per_core: int
    cap: int  # main-path edge capacity per (window, chunk) segment
    group_w: int  # windows per group (PSUM residency set)
    spill_cap: int  # spill slots per (core, chunk), multiple of 128
    gbufs: int = 2

    @property
    def chunk_rows(self) -> int:
        return self.n_nodes // NCHUNK

    @property
    def W(self) -> int:
        return -(-self.node_per_core // P)

    @property
    def n_groups(self) -> int:
        return -(-self.W // self.group_w)

    def group_windows(self, g: int) -> int:
        return min(self.group_w, self.W - g * self.group_w)

    def block_slots(self, g: int) -> int:
        # slots of one (group, chunk) gather call, padded to a tile multiple
        return -(-self.group_windows(g) * self.cap // P) * P

    def block_tiles(self, g: int) -> int:
        return self.block_slots(g) // P

    @property
    def spill_idxs(self) -> int:
        return NCHUNK * self.spill_cap

    @property
    def out_rows(self) -> int:
        return self.W * P


CFG = Cfg(n_nodes=100000, node_per_core=12500, cap=192, group_w=8,
          spill_cap=1280, gbufs=2)


def schedule(cfg: Cfg):
    """Static per-tile schedule: each gathered 128-slot tile gets ONE
    one-hot build (256 lanes wide when the tile straddles two windows) and
    one matmul per covered window. Identical on every core (SPMD).

    Returns (tiles, idx_off, spill_off, scat_off, idx_cols, oh_cols) with
    tiles = [(g, r, t, col, wlo, [(wi, start, stop), ...])]."""
    tiles = []
    col = 0
    for g in range(cfg.n_groups):
        gw = cfg.group_windows(g)
        nt = cfg.block_tiles(g)

        def wins(t):
            lo = (t * P) // cfg.cap
            hi = min((t * P + P - 1) // cfg.cap, gw - 1)
            return list(range(lo, hi + 1))

        touches = [(r, t, wi)
                   for r in range(NCHUNK)
                   for t in range(nt)
                   for wi in wins(t)]
        first = {}
        last = {}
        for k, (r, t, wi) in enumerate(touches):
            first.setdefault(wi, k)
            last[wi] = k
        k = 0
        for r in range(NCHUNK):
            for t in range(nt):
                mm = []
                for wi in wins(t):
                    mm.append((wi, first[wi] == k, last[wi] == k))
                    k += 1
                tiles.append((g, r, t, col, wins(t)[0], mm))
                col += 1
    # per-(g, r) idx-table column offset (int16 cols, wrapped/16)
    idx_off = {}
    off = 0
    for g in range(cfg.n_groups):
        for r in range(NCHUNK):
            idx_off[(g, r)] = off
            off += cfg.block_slots(g) // 16
    # spill gather idx blocks, one per chunk
    spill_off = {}
    for r in range(NCHUNK):
        spill_off[r] = off
        off += cfg.spill_cap // 16
    return tiles, idx_off, spill_off, off, col


def build(cfg: Cfg, reps: int = 1) -> bass.Bass:
    from concourse.library_config import mlp
    from concourse.library_overlay import lower_extended_insts

    tiles_s, idx_off, spill_off, idx_cols, oh_cols = schedule(cfg)
    assert cfg.chunk_rows <= 32767
    nc = bass.Bass(num_swdge_queues=NCHUNK, dynamic_dma_scratch_size=65536)
    x = nc.declare_dram_parameter("x", [cfg.n_nodes, D], mybir.dt.float16,
                                  isOutput=False)
    idxs = nc.declare_dram_parameter("idxs", [P, idx_cols], mybir.dt.int16,
                                     isOutput=False)
    ohpos = nc.declare_dram_parameter("ohpos", [P, oh_cols], mybir.dt.float32,
                                      isOutput=False)
    iota = nc.declare_dram_parameter("iota256", [P, 2 * P], mybir.dt.float16,
                                     isOutput=False)
    # transposed main output: [feature, node] so group writes have 2KB
    # contiguous runs per partition (full DMA rate); host transposes back
    out = nc.declare_dram_parameter("out", [D, cfg.out_rows],
                                    mybir.dt.float16, isOutput=True)
    # spill partial sums land in a separate compact zero-initialized
    # tensor (row = compact spill-dest id); host adds them back. This
    # removes any ordering between window writes and the scatter RMWs.
    spill_out = nc.declare_dram_parameter(
        "spill_out", [NCHUNK * cfg.spill_cap, D], mybir.dt.float16,
        isOutput=True)

    nc.gpsimd.load_library(mlp)
    with tile.TileContext(nc) as tc:
        with (
            tc.tile_pool(name="tabs", bufs=1) as tabs,
            tc.tile_pool(name="gbuf", bufs=cfg.gbufs) as gbuf,
            tc.tile_pool(name="ohb", bufs=4) as ohb,
            tc.tile_pool(name="psumb", bufs=1, space="PSUM") as psumb,
            tc.tile_pool(name="outb", bufs=4) as outb,
            tc.tile_pool(name="spb", bufs=2) as spb,
        ):
            # idx table loaded as one tile per group (plus one for spill)
            # so the first gather only waits for its own slice
            idxg = {}
            for g in range(cfg.n_groups):
                c0 = idx_off[(g, 0)]
                c1 = idx_off[(g, NCHUNK - 1)] + cfg.block_slots(g) // 16
                t = tabs.tile([P, c1 - c0], mybir.dt.int16,
                              name=f"idxg{g}")
                nc.sync.dma_start(out=t[:], in_=idxs[:, c0:c1])
                idxg[g] = (t, c0)
            c0s = spill_off[0]
            idxsp = tabs.tile([P, idx_cols - c0s], mybir.dt.int16)
            nc.sync.dma_start(out=idxsp[:], in_=idxs[:, c0s:])
            ohpos_sb = tabs.tile([P, oh_cols], mybir.dt.float32)
            iota_sb = tabs.tile([P, 2 * P], mybir.dt.float16)
            nc.sync.dma_start(out=ohpos_sb[:], in_=ohpos[:])
            nc.sync.dma_start(out=iota_sb[:], in_=iota[:])

            # shared registers for gather counts — a fresh to_reg per call
            # exhausts the Pool register file at this call count

            nregs = {}
            for g in range(cfg.n_groups):
                n = cfg.block_slots(g)
                if n not in nregs:
                    nregs[n] = nc.gpsimd.to_reg(n)
            if cfg.spill_cap not in nregs:
                nregs[cfg.spill_cap] = nc.gpsimd.to_reg(cfg.spill_cap)

            by_block = {}
            for (g, r, t, col, wlo, mm) in tiles_s:
                by_block.setdefault((g, r), []).append((t, col, wlo, mm))

            # Tile assigns DMASW sem lanes to Pool DMA insts round-robin in
            # SCHEDULED order (8 lanes), and the runtime locks each lane to
            # the first SWDGE queue that updates it. The scheduler may
            # reorder (it moves the spill gathers, whose only consumer is
            # the final scatter), so the only queue assignment that stays
            # consistent under any order is a single queue.
            def nextq():
                return 0

            for rep in range(reps):
                # spill gathers early (independent of main path); the
                # buffer is double-buffered so the bench reps don't
                # serialize on the previous rep's spill_out write
                spill_sb = spb.tile([P, (cfg.spill_idxs // P) * D],
                                    mybir.dt.float16, tag="spill",
                                    name="spill_sb")
                for r in range(NCHUNK):
                    t0 = r * (cfg.spill_cap // P)
                    nc.gpsimd.dma_gather(
                        spill_sb[:, t0 * D:(t0 + cfg.spill_cap // P) * D]
                        .rearrange("p (c d) -> p c d", c=cfg.spill_cap // P),
                        x[r * cfg.chunk_rows:(r + 1) * cfg.chunk_rows, :],
                        idxsp[:, spill_off[r] - c0s:
                              spill_off[r] - c0s + cfg.spill_cap // 16],
                        cfg.spill_cap,
                        nregs[cfg.spill_cap],
                        D,
                        queue_num=nextq(),
                        single_packet=False,
                    )
                for g in range(cfg.n_groups):
                    nt = cfg.block_tiles(g)
                    gw = cfg.group_windows(g)
                    chts = []
                    for r in range(NCHUNK):
                        ch = gbuf.tile([P, nt * D], mybir.dt.float16,
                                       tag=f"ch{r}"
                                       if gw == cfg.group_w else f"ch{r}t")
                        c0 = idx_off[(g, r)]
                        nc.gpsimd.dma_gather(
                            ch[:].rearrange("p (c d) -> p c d", c=nt),
                            x[r * cfg.chunk_rows:(r + 1) * cfg.chunk_rows, :],
                            idxg[g][0][:, c0 - idxg[g][1]:
                                       c0 - idxg[g][1]
                                       + cfg.block_slots(g) // 16],
                            cfg.block_slots(g),
                            nregs[cfg.block_slots(g)],
                            D,
                            queue_num=nextq(),
                            # single_packet coalesces the whole descriptor
                            # stream into one SDMA packet; beyond ~1K
                            # descriptors that wedges the DMA engine (HW
                            # hang). Multi-packet is required at this size.
                            single_packet=False,
                        )
                        chts.append(ch)
                    # one PSUM tile (= one bank: a matmul `start` zeroes the
                    # whole 2KB zero region) per window, accumulation groups
                    # interleave across banks
                    pstiles = {}
                    obg = outb.tile([P, gw * D], mybir.dt.float16,
                                    tag="obg" if gw == cfg.group_w
                                    else "obgt", name="obg")
                    ndone = 0
                    for r in range(NCHUNK):
                        for (t, col, wlo, mm) in by_block[(g, r)]:
                            wide = 2 if len(mm) > 1 else 1
                            oh = ohb.tile([P, wide * P], mybir.dt.float16,
                                          tag=f"oh{wide}", name="oh")
                            nc.vector.tensor_scalar(
                                out=oh[:],
                                in0=iota_sb[:, :wide * P],
                                scalar1=ohpos_sb[:, col:col + 1],
                                scalar2=None,
                                op0=mybir.AluOpType.is_equal,
                            )
                            for j, (wi, st, sp) in enumerate(mm):
                                if wi not in pstiles:
                                    pstiles[wi] = psumb.tile(
                                        [P, D], mybir.dt.float32,
                                        tag=f"ps{wi}", name=f"ps{wi}")
                                # lhsT = messages, rhs = one-hot: the PSUM
                                # comes out [feature, lane] so the group
                                # write below is a contiguous-run DMA into
                                # the transposed out
                                nc.tensor.matmul(
                                    pstiles[wi][:],
                                    lhsT=chts[r][:, t * D:(t + 1) * D],
                                    rhs=oh[:, j * P:(j + 1) * P],
                                    start=st,
                                    stop=sp,
                                )
                                if sp:
                                    nc.scalar.copy(
                                        out=obg[:, wi * D:(wi + 1) * D],
                                        in_=pstiles[wi][:])
                                    ndone += 1
                    assert ndone == gw
                    row0 = g * cfg.group_w * P
                    nc.sync.dma_start(
                        out=out[:, row0:row0 + gw * P], in_=obg[:]
                    )
                # device-computed spill partials out to the compact DRAM
                # tensor; the host adds them into the final result
                nc.sync.dma_start(
                    out=spill_out[:, :]
                    .rearrange("(c p) d -> p c d", p=P),
                    in_=spill_sb[:]
                    .rearrange("p (c d) -> p c d", c=cfg.spill_idxs // P),
                )
    rebalance_swdge_queues(nc)
    split_multi_waits(nc)
    lower_extended_insts(nc)
    return nc


def rebalance_swdge_queues(nc: "bass.Bass") -> None:
    """Spread SWDGE desc-gen across the 4 Q7 queue pairs.

    All Pool DMAs are emitted on queue 0 because Tile assigns DMASW sem
    lanes round-robin in SCHEDULED order and the runtime locks each lane
    to one queue — emission-order queue cycling breaks when the scheduler
    reorders. After scheduling, each instruction carries its lane
    (bass_scheduled_proc = DMASW<i>), so queue = i % NCHUNK is consistent
    with every lane lock while restoring 4-queue parallel desc-gen on HW.
    Post-drain instructions (the chained spill scatters) have no lane and
    keep queue 0."""
    from concourse.tile_sem_assignment import PROC_NAME_TO_IDX

    lane_q = {
        PROC_NAME_TO_IDX[f"DMASW{i}"]: i % NCHUNK for i in range(8)
    }
    for fn in nc.m.functions:
        for bb in fn.blocks:
            for inst in bb.instructions:
                if isinstance(
                    inst, (mybir.InstDMAGatherAnt, mybir.InstDMAScatterAddAnt)
                ):
                    proc = getattr(inst, "bass_scheduled_proc", None)
                    if proc in lane_q:
                        inst.queue_num = lane_q[proc]


def prep_core(row, col, node_base, cfg: Cfg):
    """Slot assignment for one core.

    Returns (idxs int16, ohpos f32, spill_slots, spill_nodes): the first
    two are device tables; the last two tell the host which spill_out row
    (device-computed message) to add into which local out row."""
    tiles_s, idx_off, spill_off, idx_cols, oh_cols = schedule(cfg)
    lo, hi = node_base, node_base + cfg.node_per_core
    m = (col >= lo) & (col < hi)
    lcol = (col[m] - lo).astype(np.int64)
    lrow = row[m].astype(np.int64)

    w = lcol >> 7
    lane = lcol & 127
    ck = lrow // cfg.chunk_rows
    lidx = lrow - ck * cfg.chunk_rows

    key = w * NCHUNK + ck
    order = np.argsort(key, kind="stable")
    key_s = key[order]
    lane_s = lane[order]
    lidx_s = lidx[order]
    lcol_s = lcol[order]

    nseg = cfg.W * NCHUNK
    counts = np.bincount(key_s, minlength=nseg)
    starts = np.zeros(nseg, np.int64)
    np.cumsum(counts[:-1], out=starts[1:])
    rank = np.arange(len(key_s)) - starts[key_s]

    main = rank < cfg.cap
    # ---- main path tables ----
    w_m = key_s[main] // NCHUNK
    r_m = key_s[main] % NCHUNK
    g_m = w_m // cfg.group_w
    wi_m = w_m - g_m * cfg.group_w
    rank_m = rank[main]
    # slot within the (g, r) call
    slot_m = wi_m * cfg.cap + rank_m

    # per-call int16 source index arrays (slot order i = t*128 + p)
    call_srcidx = {}
    for g in range(cfg.n_groups):
        for r in range(NCHUNK):
            call_srcidx[(g, r)] = np.zeros(cfg.block_slots(g), np.int16)
    # assign
    for g in range(cfg.n_groups):
        for r in range(NCHUNK):
            sel = (g_m == g) & (r_m == r)
            call_srcidx[(g, r)][slot_m[sel]] = lidx_s[main][sel]

    # ohpos [128, oh_cols]: one column per gathered tile; straddle tiles
    # encode lane' = lane + 128 * (window - first window of the tile)
    ohpos = np.full((P, oh_cols), -1.0, np.float32)
    lane_full = {}
    win_full = {}
    for g in range(cfg.n_groups):
        for r in range(NCHUNK):
            ln = np.full(cfg.block_slots(g), -1.0, np.float32)
            wn = np.full(cfg.block_slots(g), -1, np.int64)
            sel = (g_m == g) & (r_m == r)
            ln[slot_m[sel]] = lane_s[main][sel]
            wn[slot_m[sel]] = wi_m[sel]
            lane_full[(g, r)] = ln
            win_full[(g, r)] = wn
    for (g, r, t, colx, wlo, mm) in tiles_s:
        seg = slice(t * P, (t + 1) * P)
        ln = lane_full[(g, r)][seg].copy()
        wn = win_full[(g, r)][seg]
        occ = wn >= 0
        ln[occ] += 128.0 * (wn[occ] - wlo)
        ohpos[:, colx] = ln

    # ---- spill path tables ----
    sp_sel = ~main
    r_sp = key_s[sp_sel] % NCHUNK
    lidx_sp = lidx_s[sp_sel]
    lcol_sp = lcol_s[sp_sel]
    spill_src = {}
    spill_slots = []
    spill_nodes = []
    for r in range(NCHUNK):
        selr = r_sp == r
        n = int(selr.sum())
        if n > cfg.spill_cap:
            raise ValueError(f"spill overflow: {n} > {cfg.spill_cap}")
        s = np.zeros(cfg.spill_cap, np.int16)
        s[:n] = lidx_sp[selr]
        spill_src[r] = s
        spill_slots.append(r * cfg.spill_cap + np.arange(n))
        spill_nodes.append(lcol_sp[selr])
    spill_slots = np.concatenate(spill_slots)
    spill_nodes = np.concatenate(spill_nodes)

    # ---- wrapped int16 idx table ----
    def wrap(flat):
        ci16 = len(flat) // 16
        return np.tile(flat.reshape(ci16, 16).T, (8, 1))

    idxs = np.zeros((P, idx_cols), np.int16)
    for g in range(cfg.n_groups):
        for r in range(NCHUNK):
            c0 = idx_off[(g, r)]
            blk = call_srcidx[(g, r)]
            idxs[:, c0:c0 + len(blk) // 16] = wrap(blk)
    for r in range(NCHUNK):
        c0 = spill_off[r]
        idxs[:, c0:c0 + cfg.spill_cap // 16] = wrap(spill_src[r])
    return idxs, ohpos, spill_slots, spill_nodes


def prep_all(x, edge_index, cfg: Cfg):
    row = np.asarray(edge_index[0])
    col = np.asarray(edge_index[1])
    xf = np.ascontiguousarray(np.asarray(x).astype(np.float16))
    it = np.tile(np.arange(2 * P, dtype=np.float16), (P, 1))
    in_maps = []
    spill_adds = []
    for c in range(N_CORES):
        idxs, ohpos, sslots, snodes = prep_core(
            row, col, c * cfg.node_per_core, cfg)
        in_maps.append({"x": xf, "idxs": idxs, "ohpos": ohpos,
                        "iota256": it})
        spill_adds.append((sslots, snodes))
    return in_maps, spill_adds


class SpmdRunner:
    """PJRT SPMD runner for a prebuilt Bass module.

    Mirrors bass2jax.run_bass_via_pjrt but stages inputs with per-device
    device_put + make_array_from_single_device_arrays and reads outputs
    shard-by-shard: no host<->global-array slicing ops get compiled (this
    toolchain's penguin DataLocalityOpt rejects them for large arrays).
    """

    def __init__(self, nc: bass.Bass, n_cores: int = N_CORES):
        bass2jax.install_neuronx_cc_hook()
        self.nc = nc
        self.n_cores = n_cores
        pname = nc.partition_id_tensor.name if nc.partition_id_tensor else None
        self.partition_name = pname
        in_names, out_names, out_avals = [], [], []
        for alloc in nc.m.functions[0].allocations:
            if not isinstance(alloc, mybir.MemoryLocationSet):
                continue
            name = alloc.memorylocations[0].name
            if alloc.kind == "ExternalInput":
                if name != pname:
                    in_names.append(name)
            elif alloc.kind == "ExternalOutput":
                out_names.append(name)
                out_avals.append(
                    jax.core.ShapedArray(
                        tuple(alloc.tensor_shape), mybir.dt.np(alloc.dtype)
                    )
                )
        self.in_names = in_names
        self.out_names = out_names
        self.out_avals = out_avals
        self.devices = jax.devices()[:n_cores]
        self.mesh = Mesh(np.asarray(self.devices), ("core",))
        self.sharding = NamedSharding(self.mesh, PartitionSpec("core"))
        all_in_names = list(in_names) + list(out_names)
        if pname is not None:
            all_in_names.append(pname)

        def _body(*args):
            operands = list(args)
            if pname is not None:
                operands.append(bass2jax.partition_id_tensor())
            return tuple(
                bass2jax._bass_exec_p.bind(
                    *operands,
                    out_avals=tuple(out_avals),
                    in_names=tuple(all_in_names),
                    out_names=tuple(out_names),
                    lowering_input_output_aliases=(),
                    sim_require_finite=True,
                    sim_require_nnan=True,
                    nc=nc,
                )
            )

        n_args = len(in_names) + len(out_names)
        self.fn = jax.jit(
            shard_map(
                _body,
                mesh=self.mesh,
                in_specs=(PartitionSpec("core"),) * n_args,
                out_specs=(PartitionSpec("core"),) * len(out_names),
                check_rep=False,
            ),
            keep_unused=True,
        )

    def _global(self, per_core_arrays):
        shape = per_core_arrays[0].shape
        gshape = (self.n_cores * shape[0],) + tuple(shape[1:])
        bufs = [
            jax.device_put(a, d)
            for a, d in zip(per_core_arrays, self.devices)
        ]
        return jax.make_array_from_single_device_arrays(
            gshape, self.sharding, bufs
        )

    def stage(self, in_maps):
        args = [
            self._global([np.asarray(m[name]) for m in in_maps])
            for name in self.in_names
        ]
        args += [
            self._global(
                [np.zeros(av.shape, av.dtype) for _ in range(self.n_cores)]
            )
            for av in self.out_avals
        ]
        return args

    def run(self, args):
        outs = self.fn(*args)
        jax.block_until_ready(outs)
        return outs

    def to_numpy(self, outs):
        res = [dict() for _ in range(self.n_cores)]
        for i, name in enumerate(self.out_names):
            shards = sorted(
                outs[i].addressable_shards,
                key=lambda s: s.index[0].start or 0,
            )
            assert len(shards) == self.n_cores
            for c, s in enumerate(shards):
                res[c][name] = np.asarray(s.data)
        return res

    def __call__(self, in_maps):
        return self.to_numpy(self.run(self.stage(in_maps)))


_NC_CACHE = {}
_RUNNER_CACHE = {}


def _get_nc(cfg: Cfg) -> bass.Bass:
    nc = _NC_CACHE.get(cfg)
    if nc is None:
        nc = build(cfg)
        _NC_CACHE[cfg] = nc
    return nc


def _get_runner(cfg: Cfg) -> SpmdRunner:
    r = _RUNNER_CACHE.get(cfg)
    if r is None:
        r = SpmdRunner(_get_nc(cfg))
        _RUNNER_CACHE[cfg] = r
    return r


def kernel(x: np.ndarray, edge_index: np.ndarray) -> np.ndarray:
    x = np.asarray(x)
    edge_index = np.asarray(edge_index)
    try:
        in_maps, spill_adds = prep_all(x, edge_index, CFG)
    except ValueError:
        # Spill-capacity overflow (an edge distribution far from this
        # problem's uniform random graph): fall back to a host computation
        # rather than returning wrong results.
        out = np.zeros((x.shape[0], x.shape[1]), np.float32)
        np.add.at(
            out,
            np.asarray(edge_index[1], np.int64),
            np.asarray(x, np.float32)[np.asarray(edge_index[0], np.int64)],
        )
        return out
    res = _get_runner(CFG)(in_maps)
    parts = []
    for c in range(N_CORES):
        out_c = np.ascontiguousarray(
            res[c]["out"].T[: CFG.node_per_core]).astype(np.float32)
        sslots, snodes = spill_adds[c]
        if len(snodes):
            vals = res[c]["spill_out"].astype(np.float32)
            # device-computed spill messages merged by duplicate-rank round
            # so each fancy-index add has unique destination rows
            o = np.argsort(snodes, kind="stable")
            ns, ss = snodes[o], sslots[o]
            grp_start = np.ones(len(ns), bool)
            grp_start[1:] = ns[1:] != ns[:-1]
            gid = np.cumsum(grp_start) - 1
            firsts = np.nonzero(grp_start)[0]
            rnd = np.arange(len(ns)) - firsts[gid]
            for k in range(int(rnd.max()) + 1 if len(ns) else 0):
                selk = rnd == k
                out_c[ns[selk]] += vals[ss[selk]]
        parts.append(out_c)
    return np.concatenate(parts)
